# revision 21
# baseline (speedup 1.0000x reference)
"""Trainium2 Bass kernel for nn_DeepSetAttentionModel (segment_reduce).

Strategy (pure data parallel, 8 NeuronCores):
- Host sorts the 64 set rows by length (desc) and assigns rank k to
  core k%8, slot k//8. All cores run ONE SPMD program whose per-slot
  token counts are the max length within the slot's 8 rows.
- KEY ALGEBRA: the psi-MLP / masked-mean / rho_attn "agg" branch of the
  reference only adds a per-(row,head) CONSTANT to preattn, which the
  per-row softmax cancels exactly. It is dead code and is not computed.
  (Verified vs reference: diff ~1e-16.)
- preattn = Vx.T @ x per head with Vx = W_k[:32]@W_q/sqrt(DP) computed
  on host. The demo token's phi-input encoding is host-computed (tiny).
- Per core, rows are processed in 128-token chunks. Each group of 4
  chunks is transposed in ONE PE transpose into a 32-row-stacked
  feature-major layout; phi layer 1 and preattn then run as 4
  concurrent row-tiled (tile_position) matmuls. phi layer 3 is emitted
  TOKEN-major (lhsT = h2 chunk) so no enc transpose is needed.
  Invalid tokens are dropped by multiplying e=exp(preattn) with the
  valid mask before the head_agg/Z accumulation.
"""
import numpy as np

B, T = 64, 4096
CH = 128
NPOS, V, NMOD = 16, 1, 15
DP, H = 64, 4
MAXTS = 100.0
NCORES = 8
NSLOTS = B // NCORES
CMAX = 32


def _host_constants():
    ts = MAXTS ** np.linspace(0.0, 1.0, NPOS // 2).astype(np.float32)
    twopi = 2.0 * np.pi
    freq = (np.concatenate([1.0 / ts, 1.0 / ts]) / twopi).astype(np.float32)  # turns
    phase = np.array([0.0] * 8 + [0.25] * 8, np.float32)
    itp = np.concatenate([freq, phase])[None, :].repeat(128, 0)          # [128,32]
    iota15 = np.arange(NMOD, dtype=np.float32)[None, :].repeat(128, 0)   # [128,15]
    pbc = np.arange(128, dtype=np.float32)[:, None].repeat(CMAX + 1, 1)  # [128,33]
    cbc = np.arange(CMAX + 1, dtype=np.float32)[None, :].repeat(128, 0)  # [128,33]
    p0 = (np.arange(128) == 0).astype(np.float32)[:, None]               # [128,1]
    ident = np.eye(128, dtype=np.float32)
    return itp, iota15, pbc, cbc, p0, ident


# f32 const blob layout: name -> (col_offset, ncols)
F32_COLS = [("itp", 32), ("pbc", CMAX + 1), ("cbc", CMAX + 1), ("iota15", NMOD),
            ("lens", NSLOTS), ("p0", 1), ("b1", 1), ("b2", 1),
            ("rb1", 1), ("rb2", 1), ("rb3", 1), ("rw2", 128), ("rw3", 1)]
BF_COLS = [("ident", 128), ("w1stk", 128), ("w2", 128), ("w3", 128),
           ("rw1", 512), ("vxstk", H), ("demoT", NSLOTS)]


def _offsets(cols):
    out, o = {}, 0
    for n, w in cols:
        out[n] = (o, w)
        o += w
    return out, o


F32_OFF, F32_N = _offsets(F32_COLS)
BF_OFF, BF_N = _offsets(BF_COLS)


def _build_nc(Cs, has_b3, tile_mod, bass, mybir):
    """Build the SPMD program for per-slot chunk counts Cs (len 8)."""
    f32 = mybir.dt.float32
    bf16 = mybir.dt.bfloat16
    Alu = mybir.AluOpType
    Act = mybir.ActivationFunctionType
    Cmax = max(Cs)
    TWOPI = float(2.0 * np.pi)

    nc = bass.Bass()
    dt_in = {}

    def din(name, shape, dtype=f32):
        dt_in[name] = nc.dram_tensor(name, list(shape), dtype, kind="ExternalInput")
        return dt_in[name]

    d_pack = din("pack_r", [NSLOTS, 3, T])
    din("cf32", [128, F32_N])
    din("cbf", [128, BF_N], bf16)
    if has_b3:
        din("b3row", [1, 512], bf16)
        din("ones1", [1, 128], bf16)
    d_out = nc.dram_tensor("out", [NSLOTS, 1], f32, kind="ExternalOutput")

    from contextlib import ExitStack
    with tile_mod.TileContext(nc) as tc, ExitStack() as stack:
        cp = stack.enter_context(tc.tile_pool(name="const", bufs=1))
        sp = stack.enter_context(tc.tile_pool(name="sbuf", bufs=1))
        pp = stack.enter_context(tc.tile_pool(name="psum", bufs=1, space="PSUM"))

        cf32 = cp.tile([128, F32_N], f32, tag="cf32", name="cf32")
        nc.sync.dma_start(out=cf32[:], in_=dt_in["cf32"][:])
        cbf = cp.tile([128, BF_N], bf16, tag="cbf", name="cbf")
        nc.gpsimd.dma_start(out=cbf[:], in_=dt_in["cbf"][:])

        def f32c(name):
            o, w = F32_OFF[name]
            return cf32[:, o:o + w]

        def bfc(name):
            o, w = BF_OFF[name]
            return cbf[:, o:o + w]

        itp, pbc, cbc, iota15 = f32c("itp"), f32c("pbc"), f32c("cbc"), f32c("iota15")
        lensb, p0m = f32c("lens"), f32c("p0")
        b1, b2 = f32c("b1"), f32c("b2")
        ident, w1stk, w2, w3 = bfc("ident"), bfc("w1stk"), bfc("w2"), bfc("w3")
        vxstk, demoT = bfc("vxstk"), bfc("demoT")
        if has_b3:
            b3row = cp.tile([1, 512], bf16, tag="b3row", name="b3row")
            nc.sync.dma_start(out=b3row[:], in_=dt_in["b3row"][:])
            ones1 = cp.tile([1, 128], bf16, tag="ones1", name="ones1")
            nc.sync.dma_start(out=ones1[:], in_=dt_in["ones1"][:])

        feat_all = sp.tile([128, NSLOTS, H], bf16, tag="feat_all", name="feat_all")

        # HAM warm-up primer: ~7us of dense junk matmuls during the otherwise
        # idle startup (DMA/featurization) to flip the PE clock-gate to 2.4GHz
        for _ in range(16):
            ps_w = pp.tile([128, 512], f32, tag="mlp", bufs=2, name="ps_warm")
            nc.tensor.matmul(ps_w[:], ident[:], cbf[:, 0:512])

        def featurize(r):
            C = Cs[r]
            Tp = C * CH
            pk = sp.tile([128, 3, Cmax], f32, tag="pack", bufs=3, name="pk")
            nc.gpsimd.dma_start(out=pk[:, :, 0:C],
                                in_=d_pack[r, :, 0:Tp].rearrange("f (p c) -> p f c", c=C))
            times_sb = pk[:, 0, 0:C]
            vals_sb = pk[:, 1, 0:C]
            measf = pk[:, 2, 0:C]
            iot = sp.tile([128, Cmax], f32, tag="iot", bufs=3, name="iot")
            nc.vector.scalar_tensor_tensor(out=iot[:, 0:C], in0=pbc[:, 0:C],
                                           scalar=float(C), in1=cbc[:, 0:C],
                                           op0=Alu.mult, op1=Alu.add)
            # featurize token-major: xtok [128, C, 33] bf16 (ch 32 = invalid bit)
            xtok = sp.tile([128, Cmax, 33], bf16, tag="xtok", bufs=4, name="xtok")
            nc.vector.tensor_scalar(xtok[:, 0:C, 32:33], iot[:, 0:C].unsqueeze(2),
                                    lensb[:, r:r + 1], None, Alu.is_ge)
            ang = sp.tile([128, Cmax, 8], f32, tag="ang", bufs=3, name="ang")
            nc.vector.tensor_tensor(
                out=ang[:, 0:C, :],
                in0=times_sb.unsqueeze(2).to_broadcast([128, C, 8]),
                in1=itp[:, 0:8].unsqueeze(1).to_broadcast([128, C, 8]),
                op=Alu.mult)
            # round-to-nearest via f32 magic number; frac in [-0.5, 0.5]
            rnd = sp.tile([128, Cmax, 8], f32, tag="rnd", bufs=3, name="rnd")
            BIG = float(1.5 * 2 ** 23)
            nc.vector.tensor_scalar(rnd[:, 0:C, :], ang[:, 0:C, :], BIG, -BIG,
                                    Alu.add, Alu.add)
            nc.vector.tensor_tensor(out=ang[:, 0:C, :], in0=ang[:, 0:C, :],
                                    in1=rnd[:, 0:C, :], op=Alu.subtract)
            # cos half: frac2 = (frac + 0.25) - round(frac + 0.25), also reduced
            nc.vector.tensor_scalar(rnd[:, 0:C, :], ang[:, 0:C, :],
                                    float(0.25) + BIG, -BIG, Alu.add, Alu.add)
            nc.vector.scalar_tensor_tensor(out=rnd[:, 0:C, :], in0=ang[:, 0:C, :],
                                           scalar=0.25, in1=rnd[:, 0:C, :],
                                           op0=Alu.add, op1=Alu.subtract)
            nc.scalar.activation(xtok[:, 0:C, 0:8], ang[:, 0:C, :], Act.Sin,
                                 scale=TWOPI)
            nc.scalar.activation(xtok[:, 0:C, 8:16], rnd[:, 0:C, :], Act.Sin,
                                 scale=TWOPI)
            nc.vector.tensor_copy(xtok[:, 0:C, 16:17], vals_sb.unsqueeze(2))
            nc.vector.tensor_tensor(
                out=xtok[:, 0:C, 17:32],
                in0=measf.unsqueeze(2).to_broadcast([128, C, NMOD]),
                in1=iota15[:].unsqueeze(1).to_broadcast([128, C, NMOD]),
                op=Alu.is_equal)
            return xtok

        def process_mlp(r, xtok):
            C = Cs[r]
            CE = C + 1                      # + demo chunk
            Tp = C * CH
            Text = CE * CH
            NG2 = (CE + 3) // 4             # groups incl demo chunk

            xT = sp.tile([33, (Cmax + 1) * CH], bf16, tag="xT", bufs=2, name="xT")
            nc.gpsimd.memset(xT[0:32, Tp:Text], 0.0)
            nc.gpsimd.memset(xT[32:33, Tp:Text], 1.0)
            nc.vector.tensor_copy(xT[:, Tp:Tp + 1], demoT[0:33, r:r + 1])
            enc_tok = sp.tile([128, Cmax + 1, 132], bf16, tag="enc_tok", bufs=2,
                              name="enc_tok")
            ps_pre = pp.tile([128, Cmax + 1, 4], f32, tag="pre", bufs=2, name="ps_pre")
            for g in range(NG2):
                c0 = g * 4
                nch = min(4, CE - c0)       # chunks incl demo
                ndata = max(0, min(4, C - c0))
                if ndata > 0:
                    pxp = pp.tile([128, 512], bf16, tag="xpose", bufs=2, name="pxp")
                    for j in range(ndata):
                        nc.tensor.transpose(pxp[0:33, j * CH:(j + 1) * CH],
                                            xtok[:, c0 + j, :], ident[:])
                    if g % 2 == 0:
                        nc.vector.tensor_copy(xT[:, c0 * CH:(c0 + ndata) * CH],
                                              pxp[0:33, 0:ndata * CH])
                    else:
                        nc.scalar.copy(xT[:, c0 * CH:(c0 + ndata) * CH],
                                       pxp[0:33, 0:ndata * CH])
                N = nch * CH
                ps1 = pp.tile([128, 512], f32, tag="mlp", bufs=2, name="ps1")
                nc.tensor.matmul(ps1[:, 0:N], w1stk[0:32, :],
                                 xT[0:32, c0 * CH:c0 * CH + N])
                for j in range(nch):
                    nc.tensor.matmul(ps_pre[:, c0 + j, :],
                                     xT[:, (c0 + j) * CH:(c0 + j + 1) * CH],
                                     vxstk[0:33, :])
                h1 = sp.tile([128, 512], bf16, tag="h1", bufs=3, name="h1")
                nc.scalar.activation(h1[:, 0:N], ps1[:, 0:N], Act.Relu, bias=b1)
                ps2 = pp.tile([128, 512], f32, tag="mlp", bufs=2, name="ps2")
                nc.tensor.matmul(ps2[:, 0:N], w2[:], h1[:, 0:N])
                h2 = sp.tile([128, 512], bf16, tag="h2", bufs=3, name="h2")
                nc.vector.tensor_scalar(h2[:, 0:N], ps2[:, 0:N], b2, 0.0,
                                        Alu.add, Alu.max)
                ps3 = pp.tile([128, 4, 128], f32, tag="enc", bufs=2, name="ps3")
                if has_b3:
                    nc.tensor.matmul(ps3[:, 0:nch, :], ones1[:], b3row[:, 0:N],
                                     start=True, stop=False)
                for j in range(nch):
                    nc.tensor.matmul(ps3[:, j, :], h2[:, j * CH:(j + 1) * CH], w3[:],
                                     start=not has_b3, stop=True)
                if g % 2 == 0:
                    nc.scalar.activation(enc_tok[:, c0:c0 + nch, 0:128],
                                         ps3[:, 0:nch, :], Act.Relu)
                else:
                    nc.vector.tensor_scalar(enc_tok[:, c0:c0 + nch, 0:128],
                                            ps3[:, 0:nch, :], 0.0, None, Alu.max)

            return enc_tok, ps_pre

        def process_attn(r, state):
            C = Cs[r]
            CE = C + 1
            enc_tok, ps_pre = state
            # softmax-weighted segment sum + Z via ones channel
            nc.gpsimd.memset(enc_tok[:, 0:CE, 128:129], 1.0)
            e_tok = sp.tile([128, Cmax + 1, H], bf16, tag="e_tok", bufs=2, name="e_tok")
            nc.scalar.activation(e_tok[:, 0:CE, :], ps_pre[:, 0:CE, :], Act.Exp)
            ps_hh = pp.tile([4, 132], f32, tag="pre", bufs=2, name="ps_hh")
            for c in range(CE):
                nc.tensor.matmul(ps_hh[:, 0:129], e_tok[:, c, :], enc_tok[:, c, 0:129],
                                 start=(c == 0), stop=(c == CE - 1))
            rzn = sp.tile([H, 1], f32, tag="rz", bufs=2, name="rzn")
            nc.vector.reciprocal(rzn[:], ps_hh[:, 128:129])
            hh_sb = sp.tile([H, 128], bf16, tag="hh_sb", bufs=2, name="hh_sb")
            nc.vector.tensor_scalar(hh_sb[:], ps_hh[:, 0:128], rzn[:], None, Alu.mult)
            ps_tr = pp.tile([128, 4], bf16, tag="xpose", bufs=2, name="ps_tr")
            nc.tensor.transpose(ps_tr[:], hh_sb[:], ident[0:H, 0:H])
            nc.vector.tensor_copy(feat_all[:, r, :], ps_tr[:])

        # big rows first; featurize one row ahead
        xt_prev = featurize(0)
        for r in range(NSLOTS):
            xt_cur = xt_prev
            if r + 1 < NSLOTS:
                xt_prev = featurize(r + 1)
            st = process_mlp(r, xt_cur)
            process_attn(r, st)

        # ---- rho MLP over all 8 rows (f32) ----
        rw1 = bfc("rw1")
        ps_r1 = pp.tile([128, 512], f32, tag="mlp", bufs=2, name="ps_r1")
        for h in range(H):
            nc.tensor.matmul(ps_r1[:, 0:NSLOTS], rw1[:, 128 * h:128 * (h + 1)],
                             feat_all[:, :, h], start=(h == 0), stop=(h == H - 1))
        r1 = sp.tile([128, NSLOTS], f32, tag="r1", name="r1")
        nc.scalar.activation(r1[:], ps_r1[:, 0:NSLOTS], Act.Relu, bias=f32c("rb1"))
        ps_r2 = pp.tile([128, 512], f32, tag="mlp", bufs=2, name="ps_r2")
        nc.tensor.matmul(ps_r2[:, 0:NSLOTS], f32c("rw2"), r1[:])
        r2 = sp.tile([128, NSLOTS], f32, tag="r2", name="r2")
        nc.scalar.activation(r2[:], ps_r2[:, 0:NSLOTS], Act.Relu, bias=f32c("rb2"))
        ps_r3 = pp.tile([4, 512], f32, tag="pre", bufs=2, name="ps_r3")
        nc.tensor.matmul(ps_r3[0:1, 0:NSLOTS], f32c("rw3"), r2[:])
        res = sp.tile([1, NSLOTS], f32, tag="res", name="res")
        nc.scalar.activation(res[:], ps_r3[0:1, 0:NSLOTS], Act.Sigmoid,
                             bias=f32c("rb3")[0:1, :])
        nc.sync.dma_start(out=d_out[:].rearrange("r one -> one r"), in_=res[:])
    return nc


def _patch_tile_drain(tile_mod, mybir):
    """Walrus in this env rejects >1 sync wait per instruction. Two fixes:
    1) split the Tile tail drain's waits across sequential drains;
    2) a post-pass over the final BIR that moves extra waits of ANY
       instruction onto standalone NoOps inserted just before it."""
    from concourse.vector_clock import ScopedClock
    if getattr(tile_mod.TileContext, "_drain_patched", False):
        return

    def _drain_and_barrier(self, tick_clock, wait_clock):
        nc = self.nc
        drain_inst = nc.sync.drain()
        wait_clock.add_sem_waits(drain_inst.ins, ScopedClock({None: tick_clock.global_clock}))
        si = drain_inst.ins.sync_info
        waits = list(si.on_wait or [])
        if len(waits) > 1:
            si.on_wait = waits[:1]
            for i in range(1, len(waits)):
                extra = nc.sync.drain()
                esi = extra.ins.sync_info
                if esi is None:
                    extra.ins.sync_info = mybir.SyncInfo(on_wait=waits[i:i + 1], on_update=[])
                else:
                    esi.on_wait = waits[i:i + 1]
        nc.all_engine_barrier()
        popped = nc._tile_sem_poison_stack.pop()
        assert popped is self._sem_poison
        nc.clear_and_free_semaphores(list(self.sems.allocated().values()))
        nc.all_engine_barrier()

    tile_mod.TileContext._drain_and_barrier = _drain_and_barrier

    _orig_exit = tile_mod.TileContext.__exit__

    def _exit(self, exc_type, exc_val, exc_tb):
        r = _orig_exit(self, exc_type, exc_val, exc_tb)
        if exc_type is None and getattr(tile_mod.TileContext, "_split_waits", True):
            _split_multi_waits(self.nc, mybir)
        return r

    def _split_multi_waits(nc, mybir):
        n = [0]
        for f in nc.m.functions:
            for bb in f.blocks:
                insts = bb.instructions
                out = []
                for inst in insts:
                    si = inst.sync_info
                    waits = list(si.on_wait) if (si and si.on_wait) else []
                    if len(waits) > 1:
                        for w in waits[:-1]:
                            n[0] += 1
                            nop = mybir.InstNoOp(name=f"I-ws-{n[0]}", ins=[], outs=[])
                            nop.engine = inst.engine
                            nop.sync_info = mybir.SyncInfo(on_wait=[w], on_update=[])
                            out.append(nop)
                        si.on_wait = waits[-1:]
                    out.append(inst)
                if len(out) != len(insts):
                    bb.instructions = out

    tile_mod.TileContext.__exit__ = _exit
    tile_mod.TileContext._drain_patched = True


_CACHE = {}
last_results = None


def _maybe_install_ntff_shim():
    """The image's antenv lacks axon_hooks; register the ctypes NTFF hook so
    run_bass_kernel_spmd(trace=True) can profile."""
    import sys, types
    if "antenv.axon_hooks" in sys.modules:
        return
    try:
        from trn_agent_boot.trn_boot import _ntff_profile_via_ctypes
        hook = _ntff_profile_via_ctypes("/opt/axon/libaxon_pjrt.so")
    except Exception:
        hook = None
    mod = types.ModuleType("antenv.axon_hooks")
    mod.get_axon_ntff_profile_hook = lambda: hook
    sys.modules["antenv.axon_hooks"] = mod


def _to_bf16(a):
    import ml_dtypes
    return np.asarray(a, np.float32).astype(ml_dtypes.bfloat16)


def kernel(**inputs):
    import os
    import concourse.bass as bass
    import concourse.mybir as mybir
    import concourse.tile as tile_mod
    from concourse import bass_utils

    _patch_tile_drain(tile_mod, mybir)

    inp = {k: np.asarray(v) for k, v in inputs.items()}
    pack = np.stack([inp["times"].astype(np.float32)[..., 0],
                     inp["values"].astype(np.float32)[..., 0],
                     inp["measurements"].astype(np.float32)], axis=1)         # [B,3,T]
    pack = np.ascontiguousarray(pack)
    lengths = inp["lengths"].astype(np.int64)                                # [B]
    demo = inp["demo"].astype(np.float32)

    order = np.argsort(-lengths, kind="stable")
    Cs = []
    for s in range(NSLOTS):
        ranks = order[s * NCORES:(s + 1) * NCORES]
        Cs.append(int(np.ceil(lengths[ranks].max() / CH)))

    b3 = inp["phi_b3"].astype(np.float32)
    has_b3 = bool(np.abs(b3).max() > 0)

    key = (tuple(Cs), has_b3)
    if key not in _CACHE:
        _CACHE[key] = _build_nc(Cs, has_b3, tile_mod, bass, mybir)
    nc = _CACHE[key]

    itp, iota15, pbc, cbc, p0, ident = _host_constants()

    # host: Vx and demo-token phi-input features
    Wk32 = inp["W_k"].astype(np.float32)[0:32].reshape(32, H, DP)
    Vx = np.einsum('fhd,hd->fh', Wk32, inp["W_q"].astype(np.float32)) / np.sqrt(DP)
    vxstk = np.concatenate(
        [Vx, np.full((1, H), -1e9, np.float32),
         np.zeros((95, H), np.float32)], 0)       # [128,4]; row 32 = mask wt
    w1stk = np.tile(inp["phi_w1"].astype(np.float32), (4, 1))                 # [128,128]
    demo_enc = (np.maximum(demo @ inp["demo_w1"].astype(np.float32)
                           + inp["demo_b1"].astype(np.float32), 0.0)
                @ inp["demo_w2"].astype(np.float32)
                + inp["demo_b2"].astype(np.float32))                          # [B,32]

    def blob(cols, parts):
        n = sum(w for _, w in cols)
        a = np.zeros((128, n), np.float32)
        o = 0
        for name, w in cols:
            v = parts[name]
            a[:v.shape[0], o:o + w] = v
            o += w
        return a

    f32_parts = {
        "itp": itp, "pbc": pbc, "cbc": cbc, "iota15": iota15, "p0": p0,
        "b1": inp["phi_b1"].astype(np.float32)[:, None],
        "b2": inp["phi_b2"].astype(np.float32)[:, None],
        "rb1": inp["rho_b1"].astype(np.float32)[:, None],
        "rb2": inp["rho_b2"].astype(np.float32)[:, None],
        "rb3": np.broadcast_to(inp["rho_b3"].astype(np.float32)[:, None], (128, 1)),
        "rw2": inp["rho_w2"].astype(np.float32),
        "rw3": inp["rho_w3"].astype(np.float32),
    }
    bf_parts_common = {
        "ident": ident, "w1stk": w1stk, "w2": inp["phi_w2"].astype(np.float32),
        "w3": inp["phi_w3"].astype(np.float32),
        "rw1": inp["rho_w1"].astype(np.float32).reshape(4, 128, 128)
              .transpose(1, 0, 2).reshape(128, 512),
        "vxstk": vxstk,
    }

    in_maps = []
    for core in range(NCORES):
        rows = [order[s * NCORES + core] for s in range(NSLOTS)]
        f32_parts["lens"] = np.broadcast_to(
            lengths[rows].astype(np.float32)[None, :], (128, NSLOTS))
        bf_parts = dict(bf_parts_common)
        bf_parts["demoT"] = np.concatenate(
            [demo_enc[rows].T, np.zeros((96, NSLOTS), np.float32)], 0)        # [128,8]
        m = {
            "pack_r": pack[rows],
            "cf32": blob(F32_COLS, f32_parts),
            "cbf": _to_bf16(blob(BF_COLS, bf_parts)),
        }
        if has_b3:
            m["b3row"] = _to_bf16(np.tile(b3[None, :], (1, 4)))
            m["ones1"] = _to_bf16(np.ones((1, 128), np.float32))
        in_maps.append(m)

    trace = os.environ.get("KERNEL_TRACE", "0") == "1"
    kw = {}
    if trace:
        _maybe_install_ntff_shim()
        kw = dict(trace=True, tmpdir=os.environ.get("KERNEL_TRACE_DIR") or None)
    res = bass_utils.run_bass_kernel_spmd(nc, in_maps, core_ids=list(range(NCORES)), **kw)
    global last_results
    last_results = res
    out = np.zeros((B, 1), np.float32)
    for core in range(NCORES):
        for s in range(NSLOTS):
            out[order[s * NCORES + core], 0] = res.results[core]["out"][s, 0]
    return out


# revision 22
# speedup vs baseline: 1.0736x; 1.0736x over previous
"""Trainium2 Bass kernel for nn_DeepSetAttentionModel (segment_reduce).

Strategy (pure data parallel, 8 NeuronCores):
- Host sorts the 64 set rows by length (desc) and assigns rank k to
  core k%8, slot k//8. All cores run ONE SPMD program whose per-slot
  token counts are the max length within the slot's 8 rows.
- KEY ALGEBRA: the psi-MLP / masked-mean / rho_attn "agg" branch of the
  reference only adds a per-(row,head) CONSTANT to preattn, which the
  per-row softmax cancels exactly. It is dead code and is not computed.
  (Verified vs reference: diff ~1e-16.)
- preattn = Vx.T @ x per head with Vx = W_k[:32]@W_q/sqrt(DP) computed
  on host. The demo token's phi-input encoding is host-computed (tiny).
- Per core, rows are processed in 128-token chunks. Each group of 4
  chunks is transposed in ONE PE transpose into a 32-row-stacked
  feature-major layout; phi layer 1 and preattn then run as 4
  concurrent row-tiled (tile_position) matmuls. phi layer 3 is emitted
  TOKEN-major (lhsT = h2 chunk) so no enc transpose is needed.
  Invalid tokens are dropped by multiplying e=exp(preattn) with the
  valid mask before the head_agg/Z accumulation.
"""
import numpy as np

B, T = 64, 4096
CH = 128
NPOS, V, NMOD = 16, 1, 15
DP, H = 64, 4
MAXTS = 100.0
NCORES = 8
NSLOTS = B // NCORES
CMAX = 32


def _host_constants():
    ts = MAXTS ** np.linspace(0.0, 1.0, NPOS // 2).astype(np.float32)
    twopi = 2.0 * np.pi
    freq = (np.concatenate([1.0 / ts, 1.0 / ts]) / twopi).astype(np.float32)  # turns
    phase = np.array([0.0] * 8 + [0.25] * 8, np.float32)
    itp = np.concatenate([freq, phase])[None, :].repeat(128, 0)          # [128,32]
    iota15 = np.arange(NMOD, dtype=np.float32)[None, :].repeat(128, 0)   # [128,15]
    pbc = np.arange(128, dtype=np.float32)[:, None].repeat(CMAX + 1, 1)  # [128,33]
    cbc = np.arange(CMAX + 1, dtype=np.float32)[None, :].repeat(128, 0)  # [128,33]
    p0 = (np.arange(128) == 0).astype(np.float32)[:, None]               # [128,1]
    ident = np.eye(128, dtype=np.float32)
    return itp, iota15, pbc, cbc, p0, ident


# f32 const blob layout: name -> (col_offset, ncols)
F32_COLS = [("itp", 32), ("pbc", CMAX + 1), ("cbc", CMAX + 1), ("iota15", NMOD),
            ("lens", NSLOTS), ("p0", 1), ("b1", 1), ("b2", 1),
            ("rb1", 1), ("rb2", 1), ("rb3", 1), ("rw2", 128), ("rw3", 1)]
BF_COLS = [("ident", 128), ("w1stk", 128), ("w2", 128), ("w3", 128),
           ("rw1", 512), ("vxstk", H), ("demoT", NSLOTS)]


def _offsets(cols):
    out, o = {}, 0
    for n, w in cols:
        out[n] = (o, w)
        o += w
    return out, o


F32_OFF, F32_N = _offsets(F32_COLS)
BF_OFF, BF_N = _offsets(BF_COLS)


def _build_nc(Cs, has_b3, tile_mod, bass, mybir):
    """Build the SPMD program for per-slot chunk counts Cs (len 8)."""
    f32 = mybir.dt.float32
    bf16 = mybir.dt.bfloat16
    Alu = mybir.AluOpType
    Act = mybir.ActivationFunctionType
    Cmax = max(Cs)
    TWOPI = float(2.0 * np.pi)

    nc = bass.Bass()
    dt_in = {}

    def din(name, shape, dtype=f32):
        dt_in[name] = nc.dram_tensor(name, list(shape), dtype, kind="ExternalInput")
        return dt_in[name]

    d_pack = din("pack_r", [NSLOTS, 3, T])
    din("cf32", [128, F32_N])
    din("cbf", [128, BF_N], bf16)
    if has_b3:
        din("b3row", [1, 512], bf16)
        din("ones1", [1, 128], bf16)
    d_out = nc.dram_tensor("out", [NSLOTS, 1], f32, kind="ExternalOutput")

    from contextlib import ExitStack
    with tile_mod.TileContext(nc) as tc, ExitStack() as stack:
        cp = stack.enter_context(tc.tile_pool(name="const", bufs=1))
        sp = stack.enter_context(tc.tile_pool(name="sbuf", bufs=1))
        pp = stack.enter_context(tc.tile_pool(name="psum", bufs=1, space="PSUM"))

        cf32 = cp.tile([128, F32_N], f32, tag="cf32", name="cf32")
        nc.sync.dma_start(out=cf32[:], in_=dt_in["cf32"][:])
        cbf = cp.tile([128, BF_N], bf16, tag="cbf", name="cbf")
        nc.gpsimd.dma_start(out=cbf[:], in_=dt_in["cbf"][:])

        def f32c(name):
            o, w = F32_OFF[name]
            return cf32[:, o:o + w]

        def bfc(name):
            o, w = BF_OFF[name]
            return cbf[:, o:o + w]

        itp, pbc, cbc, iota15 = f32c("itp"), f32c("pbc"), f32c("cbc"), f32c("iota15")
        lensb, p0m = f32c("lens"), f32c("p0")
        b1, b2 = f32c("b1"), f32c("b2")
        ident, w1stk, w2, w3 = bfc("ident"), bfc("w1stk"), bfc("w2"), bfc("w3")
        vxstk, demoT = bfc("vxstk"), bfc("demoT")
        if has_b3:
            b3row = cp.tile([1, 512], bf16, tag="b3row", name="b3row")
            nc.sync.dma_start(out=b3row[:], in_=dt_in["b3row"][:])
            ones1 = cp.tile([1, 128], bf16, tag="ones1", name="ones1")
            nc.sync.dma_start(out=ones1[:], in_=dt_in["ones1"][:])

        feat_all = sp.tile([128, NSLOTS, H], bf16, tag="feat_all", name="feat_all")

        # HAM warm-up primer: ~7us of dense junk matmuls during the otherwise
        # idle startup (DMA/featurization) to flip the PE clock-gate to 2.4GHz
        for _ in range(16):
            ps_w = pp.tile([128, 512], f32, tag="mlp", bufs=2, name="ps_warm")
            nc.tensor.matmul(ps_w[:], ident[:], cbf[:, 0:512])

        def featurize(r):
            C = Cs[r]
            Tp = C * CH
            pk = sp.tile([128, 3, Cmax], f32, tag="pack", bufs=3, name="pk")
            nc.gpsimd.dma_start(out=pk[:, :, 0:C],
                                in_=d_pack[r, :, 0:Tp].rearrange("f (p c) -> p f c", c=C))
            times_sb = pk[:, 0, 0:C]
            vals_sb = pk[:, 1, 0:C]
            measf = pk[:, 2, 0:C]
            iot = sp.tile([128, Cmax], f32, tag="iot", bufs=3, name="iot")
            nc.vector.scalar_tensor_tensor(out=iot[:, 0:C], in0=pbc[:, 0:C],
                                           scalar=float(C), in1=cbc[:, 0:C],
                                           op0=Alu.mult, op1=Alu.add)
            # featurize token-major: xtok [128, C, 33] bf16 (ch 32 = invalid bit)
            xtok = sp.tile([128, Cmax, 33], bf16, tag="xtok", bufs=4, name="xtok")
            nc.vector.tensor_scalar(xtok[:, 0:C, 32:33], iot[:, 0:C].unsqueeze(2),
                                    lensb[:, r:r + 1], None, Alu.is_ge)
            ang = sp.tile([128, Cmax, 8], f32, tag="ang", bufs=3, name="ang")
            nc.vector.tensor_tensor(
                out=ang[:, 0:C, :],
                in0=times_sb.unsqueeze(2).to_broadcast([128, C, 8]),
                in1=itp[:, 0:8].unsqueeze(1).to_broadcast([128, C, 8]),
                op=Alu.mult)
            # round-to-nearest via f32 magic number; frac in [-0.5, 0.5]
            rnd = sp.tile([128, Cmax, 8], f32, tag="rnd", bufs=3, name="rnd")
            BIG = float(1.5 * 2 ** 23)
            nc.vector.tensor_scalar(rnd[:, 0:C, :], ang[:, 0:C, :], BIG, -BIG,
                                    Alu.add, Alu.add)
            nc.vector.tensor_tensor(out=ang[:, 0:C, :], in0=ang[:, 0:C, :],
                                    in1=rnd[:, 0:C, :], op=Alu.subtract)
            # cos half: frac2 = (frac + 0.25) - round(frac + 0.25), also reduced
            nc.vector.tensor_scalar(rnd[:, 0:C, :], ang[:, 0:C, :],
                                    float(0.25) + BIG, -BIG, Alu.add, Alu.add)
            nc.vector.scalar_tensor_tensor(out=rnd[:, 0:C, :], in0=ang[:, 0:C, :],
                                           scalar=0.25, in1=rnd[:, 0:C, :],
                                           op0=Alu.add, op1=Alu.subtract)
            nc.scalar.activation(xtok[:, 0:C, 0:8], ang[:, 0:C, :], Act.Sin,
                                 scale=TWOPI)
            nc.scalar.activation(xtok[:, 0:C, 8:16], rnd[:, 0:C, :], Act.Sin,
                                 scale=TWOPI)
            nc.vector.tensor_copy(xtok[:, 0:C, 16:17], vals_sb.unsqueeze(2))
            nc.vector.tensor_tensor(
                out=xtok[:, 0:C, 17:32],
                in0=measf.unsqueeze(2).to_broadcast([128, C, NMOD]),
                in1=iota15[:].unsqueeze(1).to_broadcast([128, C, NMOD]),
                op=Alu.is_equal)
            return xtok

        def process_mlp(r, xtok):
            C = Cs[r]
            CE = C + 1                      # + demo chunk
            Tp = C * CH
            Text = CE * CH
            NG2 = (CE + 3) // 4             # groups incl demo chunk

            xT = sp.tile([33, (Cmax + 1) * CH], bf16, tag="xT", bufs=2, name="xT")
            nc.gpsimd.memset(xT[0:32, Tp:Text], 0.0)
            nc.gpsimd.memset(xT[32:33, Tp:Text], 1.0)
            nc.vector.tensor_copy(xT[:, Tp:Tp + 1], demoT[0:33, r:r + 1])
            enc_tok = sp.tile([128, Cmax + 1, 132], bf16, tag="enc_tok", bufs=2,
                              name="enc_tok")
            ps_pre = pp.tile([128, Cmax + 1, 4], f32, tag="pre", bufs=2, name="ps_pre")
            for g in range(NG2):
                c0 = g * 4
                nch = min(4, CE - c0)       # chunks incl demo
                ndata = max(0, min(4, C - c0))
                if ndata > 0:
                    pxp = pp.tile([128, 512], bf16, tag="xpose", bufs=2, name="pxp")
                    for j in range(ndata):
                        nc.tensor.transpose(pxp[0:33, j * CH:(j + 1) * CH],
                                            xtok[:, c0 + j, :], ident[:])
                    if g % 2 == 0:
                        nc.vector.tensor_copy(xT[:, c0 * CH:(c0 + ndata) * CH],
                                              pxp[0:33, 0:ndata * CH])
                    else:
                        nc.scalar.copy(xT[:, c0 * CH:(c0 + ndata) * CH],
                                       pxp[0:33, 0:ndata * CH])
                N = nch * CH
                ps1 = pp.tile([128, 512], f32, tag="mlp", bufs=2, name="ps1")
                nc.tensor.matmul(ps1[:, 0:N], w1stk[0:32, :],
                                 xT[0:32, c0 * CH:c0 * CH + N])
                for j in range(nch):
                    nc.tensor.matmul(ps_pre[:, c0 + j, :],
                                     xT[:, (c0 + j) * CH:(c0 + j + 1) * CH],
                                     vxstk[0:33, :])
                h1 = sp.tile([128, 512], bf16, tag="h1", bufs=3, name="h1")
                nc.scalar.activation(h1[:, 0:N], ps1[:, 0:N], Act.Relu, bias=b1)
                ps2 = pp.tile([128, 512], f32, tag="mlp", bufs=2, name="ps2")
                nc.tensor.matmul(ps2[:, 0:N], w2[:], h1[:, 0:N])
                h2 = sp.tile([128, 512], bf16, tag="h2", bufs=3, name="h2")
                nc.vector.tensor_scalar(h2[:, 0:N], ps2[:, 0:N], b2, 0.0,
                                        Alu.add, Alu.max)
                ps3 = pp.tile([128, 4, 128], f32, tag="enc", bufs=2, name="ps3")
                if has_b3:
                    nc.tensor.matmul(ps3[:, 0:nch, :], ones1[:], b3row[:, 0:N],
                                     start=True, stop=False)
                for j in range(nch):
                    nc.tensor.matmul(ps3[:, j, :], h2[:, j * CH:(j + 1) * CH], w3[:],
                                     start=not has_b3, stop=True)
                if g % 2 == 0:
                    nc.scalar.activation(enc_tok[:, c0:c0 + nch, 0:128],
                                         ps3[:, 0:nch, :], Act.Relu)
                else:
                    nc.vector.tensor_scalar(enc_tok[:, c0:c0 + nch, 0:128],
                                            ps3[:, 0:nch, :], 0.0, None, Alu.max)

            return enc_tok, ps_pre

        def process_attn(r, state):
            C = Cs[r]
            CE = C + 1
            enc_tok, ps_pre = state
            # softmax-weighted segment sum + Z via ones channel
            nc.gpsimd.memset(enc_tok[:, 0:CE, 128:129], 1.0)
            e_tok = sp.tile([128, Cmax + 1, H], bf16, tag="e_tok", bufs=2, name="e_tok")
            nc.scalar.activation(e_tok[:, 0:CE, :], ps_pre[:, 0:CE, :], Act.Exp)
            ps_hh = pp.tile([4, 132], f32, tag="pre", bufs=2, name="ps_hh")
            for c in range(CE):
                nc.tensor.matmul(ps_hh[:, 0:129], e_tok[:, c, :], enc_tok[:, c, 0:129],
                                 start=(c == 0), stop=(c == CE - 1))
            rzn = sp.tile([H, 1], f32, tag="rz", bufs=2, name="rzn")
            nc.vector.reciprocal(rzn[:], ps_hh[:, 128:129])
            hh_sb = sp.tile([H, 128], bf16, tag="hh_sb", bufs=2, name="hh_sb")
            nc.vector.tensor_scalar(hh_sb[:], ps_hh[:, 0:128], rzn[:], None, Alu.mult)
            ps_tr = pp.tile([128, 4], bf16, tag="xpose", bufs=2, name="ps_tr")
            nc.tensor.transpose(ps_tr[:], hh_sb[:], ident[0:H, 0:H])
            nc.vector.tensor_copy(feat_all[:, r, :], ps_tr[:])

        # big rows first; featurize one pair ahead; pair exp/head_agg so the
        # scalar engine switches act tables (sin<->exp) once per pair
        feats = {0: featurize(0), 1: featurize(1)}
        for p in range(0, NSLOTS, 2):
            for q in (p + 2, p + 3):
                if q < NSLOTS:
                    feats[q] = featurize(q)
            st0 = process_mlp(p, feats.pop(p))
            st1 = process_mlp(p + 1, feats.pop(p + 1))
            process_attn(p, st0)
            process_attn(p + 1, st1)

        # ---- rho MLP over all 8 rows (f32) ----
        rw1 = bfc("rw1")
        ps_r1 = pp.tile([128, 512], f32, tag="mlp", bufs=2, name="ps_r1")
        for h in range(H):
            nc.tensor.matmul(ps_r1[:, 0:NSLOTS], rw1[:, 128 * h:128 * (h + 1)],
                             feat_all[:, :, h], start=(h == 0), stop=(h == H - 1))
        r1 = sp.tile([128, NSLOTS], f32, tag="r1", name="r1")
        nc.scalar.activation(r1[:], ps_r1[:, 0:NSLOTS], Act.Relu, bias=f32c("rb1"))
        ps_r2 = pp.tile([128, 512], f32, tag="mlp", bufs=2, name="ps_r2")
        nc.tensor.matmul(ps_r2[:, 0:NSLOTS], f32c("rw2"), r1[:])
        r2 = sp.tile([128, NSLOTS], f32, tag="r2", name="r2")
        nc.scalar.activation(r2[:], ps_r2[:, 0:NSLOTS], Act.Relu, bias=f32c("rb2"))
        ps_r3 = pp.tile([4, 512], f32, tag="pre", bufs=2, name="ps_r3")
        nc.tensor.matmul(ps_r3[0:1, 0:NSLOTS], f32c("rw3"), r2[:])
        res = sp.tile([1, NSLOTS], f32, tag="res", name="res")
        nc.scalar.activation(res[:], ps_r3[0:1, 0:NSLOTS], Act.Sigmoid,
                             bias=f32c("rb3")[0:1, :])
        nc.sync.dma_start(out=d_out[:].rearrange("r one -> one r"), in_=res[:])
    return nc


def _patch_tile_drain(tile_mod, mybir):
    """Walrus in this env rejects >1 sync wait per instruction. Two fixes:
    1) split the Tile tail drain's waits across sequential drains;
    2) a post-pass over the final BIR that moves extra waits of ANY
       instruction onto standalone NoOps inserted just before it."""
    from concourse.vector_clock import ScopedClock
    if getattr(tile_mod.TileContext, "_drain_patched", False):
        return

    def _drain_and_barrier(self, tick_clock, wait_clock):
        nc = self.nc
        drain_inst = nc.sync.drain()
        wait_clock.add_sem_waits(drain_inst.ins, ScopedClock({None: tick_clock.global_clock}))
        si = drain_inst.ins.sync_info
        waits = list(si.on_wait or [])
        if len(waits) > 1:
            si.on_wait = waits[:1]
            for i in range(1, len(waits)):
                extra = nc.sync.drain()
                esi = extra.ins.sync_info
                if esi is None:
                    extra.ins.sync_info = mybir.SyncInfo(on_wait=waits[i:i + 1], on_update=[])
                else:
                    esi.on_wait = waits[i:i + 1]
        nc.all_engine_barrier()
        popped = nc._tile_sem_poison_stack.pop()
        assert popped is self._sem_poison
        nc.clear_and_free_semaphores(list(self.sems.allocated().values()))
        nc.all_engine_barrier()

    tile_mod.TileContext._drain_and_barrier = _drain_and_barrier

    _orig_exit = tile_mod.TileContext.__exit__

    def _exit(self, exc_type, exc_val, exc_tb):
        r = _orig_exit(self, exc_type, exc_val, exc_tb)
        if exc_type is None and getattr(tile_mod.TileContext, "_split_waits", True):
            _split_multi_waits(self.nc, mybir)
        return r

    def _split_multi_waits(nc, mybir):
        n = [0]
        for f in nc.m.functions:
            for bb in f.blocks:
                insts = bb.instructions
                out = []
                for inst in insts:
                    si = inst.sync_info
                    waits = list(si.on_wait) if (si and si.on_wait) else []
                    if len(waits) > 1:
                        for w in waits[:-1]:
                            n[0] += 1
                            nop = mybir.InstNoOp(name=f"I-ws-{n[0]}", ins=[], outs=[])
                            nop.engine = inst.engine
                            nop.sync_info = mybir.SyncInfo(on_wait=[w], on_update=[])
                            out.append(nop)
                        si.on_wait = waits[-1:]
                    out.append(inst)
                if len(out) != len(insts):
                    bb.instructions = out

    tile_mod.TileContext.__exit__ = _exit
    tile_mod.TileContext._drain_patched = True


_CACHE = {}
last_results = None


def _maybe_install_ntff_shim():
    """The image's antenv lacks axon_hooks; register the ctypes NTFF hook so
    run_bass_kernel_spmd(trace=True) can profile."""
    import sys, types
    if "antenv.axon_hooks" in sys.modules:
        return
    try:
        from trn_agent_boot.trn_boot import _ntff_profile_via_ctypes
        hook = _ntff_profile_via_ctypes("/opt/axon/libaxon_pjrt.so")
    except Exception:
        hook = None
    mod = types.ModuleType("antenv.axon_hooks")
    mod.get_axon_ntff_profile_hook = lambda: hook
    sys.modules["antenv.axon_hooks"] = mod


def _to_bf16(a):
    import ml_dtypes
    return np.asarray(a, np.float32).astype(ml_dtypes.bfloat16)


def kernel(**inputs):
    import os
    import concourse.bass as bass
    import concourse.mybir as mybir
    import concourse.tile as tile_mod
    from concourse import bass_utils

    _patch_tile_drain(tile_mod, mybir)

    inp = {k: np.asarray(v) for k, v in inputs.items()}
    pack = np.stack([inp["times"].astype(np.float32)[..., 0],
                     inp["values"].astype(np.float32)[..., 0],
                     inp["measurements"].astype(np.float32)], axis=1)         # [B,3,T]
    pack = np.ascontiguousarray(pack)
    lengths = inp["lengths"].astype(np.int64)                                # [B]
    demo = inp["demo"].astype(np.float32)

    order = np.argsort(-lengths, kind="stable")
    Cs = []
    for s in range(NSLOTS):
        ranks = order[s * NCORES:(s + 1) * NCORES]
        Cs.append(int(np.ceil(lengths[ranks].max() / CH)))

    b3 = inp["phi_b3"].astype(np.float32)
    has_b3 = bool(np.abs(b3).max() > 0)

    key = (tuple(Cs), has_b3)
    if key not in _CACHE:
        _CACHE[key] = _build_nc(Cs, has_b3, tile_mod, bass, mybir)
    nc = _CACHE[key]

    itp, iota15, pbc, cbc, p0, ident = _host_constants()

    # host: Vx and demo-token phi-input features
    Wk32 = inp["W_k"].astype(np.float32)[0:32].reshape(32, H, DP)
    Vx = np.einsum('fhd,hd->fh', Wk32, inp["W_q"].astype(np.float32)) / np.sqrt(DP)
    vxstk = np.concatenate(
        [Vx, np.full((1, H), -1e9, np.float32),
         np.zeros((95, H), np.float32)], 0)       # [128,4]; row 32 = mask wt
    w1stk = np.tile(inp["phi_w1"].astype(np.float32), (4, 1))                 # [128,128]
    demo_enc = (np.maximum(demo @ inp["demo_w1"].astype(np.float32)
                           + inp["demo_b1"].astype(np.float32), 0.0)
                @ inp["demo_w2"].astype(np.float32)
                + inp["demo_b2"].astype(np.float32))                          # [B,32]

    def blob(cols, parts):
        n = sum(w for _, w in cols)
        a = np.zeros((128, n), np.float32)
        o = 0
        for name, w in cols:
            v = parts[name]
            a[:v.shape[0], o:o + w] = v
            o += w
        return a

    f32_parts = {
        "itp": itp, "pbc": pbc, "cbc": cbc, "iota15": iota15, "p0": p0,
        "b1": inp["phi_b1"].astype(np.float32)[:, None],
        "b2": inp["phi_b2"].astype(np.float32)[:, None],
        "rb1": inp["rho_b1"].astype(np.float32)[:, None],
        "rb2": inp["rho_b2"].astype(np.float32)[:, None],
        "rb3": np.broadcast_to(inp["rho_b3"].astype(np.float32)[:, None], (128, 1)),
        "rw2": inp["rho_w2"].astype(np.float32),
        "rw3": inp["rho_w3"].astype(np.float32),
    }
    bf_parts_common = {
        "ident": ident, "w1stk": w1stk, "w2": inp["phi_w2"].astype(np.float32),
        "w3": inp["phi_w3"].astype(np.float32),
        "rw1": inp["rho_w1"].astype(np.float32).reshape(4, 128, 128)
              .transpose(1, 0, 2).reshape(128, 512),
        "vxstk": vxstk,
    }

    in_maps = []
    for core in range(NCORES):
        rows = [order[s * NCORES + core] for s in range(NSLOTS)]
        f32_parts["lens"] = np.broadcast_to(
            lengths[rows].astype(np.float32)[None, :], (128, NSLOTS))
        bf_parts = dict(bf_parts_common)
        bf_parts["demoT"] = np.concatenate(
            [demo_enc[rows].T, np.zeros((96, NSLOTS), np.float32)], 0)        # [128,8]
        m = {
            "pack_r": pack[rows],
            "cf32": blob(F32_COLS, f32_parts),
            "cbf": _to_bf16(blob(BF_COLS, bf_parts)),
        }
        if has_b3:
            m["b3row"] = _to_bf16(np.tile(b3[None, :], (1, 4)))
            m["ones1"] = _to_bf16(np.ones((1, 128), np.float32))
        in_maps.append(m)

    trace = os.environ.get("KERNEL_TRACE", "0") == "1"
    kw = {}
    if trace:
        _maybe_install_ntff_shim()
        kw = dict(trace=True, tmpdir=os.environ.get("KERNEL_TRACE_DIR") or None)
    res = bass_utils.run_bass_kernel_spmd(nc, in_maps, core_ids=list(range(NCORES)), **kw)
    global last_results
    last_results = res
    out = np.zeros((B, 1), np.float32)
    for core in range(NCORES):
        for s in range(NSLOTS):
            out[order[s * NCORES + core], 0] = res.results[core]["out"][s, 0]
    return out


# revision 23
# speedup vs baseline: 1.1128x; 1.0365x over previous
"""Trainium2 Bass kernel for nn_DeepSetAttentionModel (segment_reduce).

Strategy (pure data parallel, 8 NeuronCores):
- Host sorts the 64 set rows by length (desc) and assigns rank k to
  core k%8, slot k//8. All cores run ONE SPMD program whose per-slot
  token counts are the max length within the slot's 8 rows.
- KEY ALGEBRA: the psi-MLP / masked-mean / rho_attn "agg" branch of the
  reference only adds a per-(row,head) CONSTANT to preattn, which the
  per-row softmax cancels exactly. It is dead code and is not computed.
  (Verified vs reference: diff ~1e-16.)
- preattn = Vx.T @ x per head with Vx = W_k[:32]@W_q/sqrt(DP) computed
  on host. The demo token's phi-input encoding is host-computed (tiny).
- Per core, rows are processed in 128-token chunks. Each group of 4
  chunks is transposed in ONE PE transpose into a 32-row-stacked
  feature-major layout; phi layer 1 and preattn then run as 4
  concurrent row-tiled (tile_position) matmuls. phi layer 3 is emitted
  TOKEN-major (lhsT = h2 chunk) so no enc transpose is needed.
  Invalid tokens are dropped by multiplying e=exp(preattn) with the
  valid mask before the head_agg/Z accumulation.
"""
import numpy as np

B, T = 64, 4096
CH = 128
NPOS, V, NMOD = 16, 1, 15
DP, H = 64, 4
MAXTS = 100.0
NCORES = 8
NSLOTS = B // NCORES
CMAX = 32


def _host_constants():
    ts = MAXTS ** np.linspace(0.0, 1.0, NPOS // 2).astype(np.float32)
    twopi = 2.0 * np.pi
    freq = (np.concatenate([1.0 / ts, 1.0 / ts]) / twopi).astype(np.float32)  # turns
    phase = np.array([0.0] * 8 + [0.25] * 8, np.float32)
    itp = np.concatenate([freq, phase])[None, :].repeat(128, 0)          # [128,32]
    iota15 = np.arange(NMOD, dtype=np.float32)[None, :].repeat(128, 0)   # [128,15]
    pbc = np.arange(128, dtype=np.float32)[:, None].repeat(CMAX + 1, 1)  # [128,33]
    cbc = np.arange(CMAX + 1, dtype=np.float32)[None, :].repeat(128, 0)  # [128,33]
    p0 = (np.arange(128) == 0).astype(np.float32)[:, None]               # [128,1]
    ident = np.eye(128, dtype=np.float32)
    return itp, iota15, pbc, cbc, p0, ident


# f32 const blob layout: name -> (col_offset, ncols)
F32_COLS = [("itp", 32), ("pbc", CMAX + 1), ("cbc", CMAX + 1), ("iota15", NMOD),
            ("lens", NSLOTS), ("p0", 1), ("b1", 1), ("b2", 1),
            ("rb1", 1), ("rb2", 1), ("rb3", 1), ("rw2", 128), ("rw3", 1)]
BF_COLS = [("ident", 128), ("w1stk", 128), ("w2", 128), ("w3", 128),
           ("rw1", 512), ("vxstk", H), ("demoT", NSLOTS)]


def _offsets(cols):
    out, o = {}, 0
    for n, w in cols:
        out[n] = (o, w)
        o += w
    return out, o


F32_OFF, F32_N = _offsets(F32_COLS)
BF_OFF, BF_N = _offsets(BF_COLS)


def _build_nc(Cs, has_b3, tile_mod, bass, mybir):
    """Build the SPMD program for per-slot chunk counts Cs (len 8)."""
    f32 = mybir.dt.float32
    bf16 = mybir.dt.bfloat16
    Alu = mybir.AluOpType
    Act = mybir.ActivationFunctionType
    Cmax = max(Cs)
    TWOPI = float(2.0 * np.pi)

    nc = bass.Bass()
    dt_in = {}

    def din(name, shape, dtype=f32):
        dt_in[name] = nc.dram_tensor(name, list(shape), dtype, kind="ExternalInput")
        return dt_in[name]

    d_pack = din("pack_r", [NSLOTS, 3, T])
    din("cf32", [128, F32_N])
    din("cbf", [128, BF_N], bf16)
    if has_b3:
        din("b3row", [1, 512], bf16)
        din("ones1", [1, 128], bf16)
    d_out = nc.dram_tensor("out", [NSLOTS, 1], f32, kind="ExternalOutput")

    from contextlib import ExitStack
    with tile_mod.TileContext(nc) as tc, ExitStack() as stack:
        cp = stack.enter_context(tc.tile_pool(name="const", bufs=1))
        sp = stack.enter_context(tc.tile_pool(name="sbuf", bufs=1))
        pp = stack.enter_context(tc.tile_pool(name="psum", bufs=1, space="PSUM"))

        cf32 = cp.tile([128, F32_N], f32, tag="cf32", name="cf32")
        nc.sync.dma_start(out=cf32[:], in_=dt_in["cf32"][:])
        cbf = cp.tile([128, BF_N], bf16, tag="cbf", name="cbf")
        nc.sync.dma_start(out=cbf[:], in_=dt_in["cbf"][:])

        def f32c(name):
            o, w = F32_OFF[name]
            return cf32[:, o:o + w]

        def bfc(name):
            o, w = BF_OFF[name]
            return cbf[:, o:o + w]

        itp, pbc, cbc, iota15 = f32c("itp"), f32c("pbc"), f32c("cbc"), f32c("iota15")
        lensb, p0m = f32c("lens"), f32c("p0")
        b1, b2 = f32c("b1"), f32c("b2")
        ident, w1stk, w2, w3 = bfc("ident"), bfc("w1stk"), bfc("w2"), bfc("w3")
        vxstk, demoT = bfc("vxstk"), bfc("demoT")
        if has_b3:
            b3row = cp.tile([1, 512], bf16, tag="b3row", name="b3row")
            nc.sync.dma_start(out=b3row[:], in_=dt_in["b3row"][:])
            ones1 = cp.tile([1, 128], bf16, tag="ones1", name="ones1")
            nc.sync.dma_start(out=ones1[:], in_=dt_in["ones1"][:])

        feat_all = sp.tile([128, NSLOTS, H], bf16, tag="feat_all", name="feat_all")

        # HAM warm-up primer: ~7us of dense junk matmuls during the otherwise
        # idle startup (DMA/featurization) to flip the PE clock-gate to 2.4GHz
        for _ in range(16):
            ps_w = pp.tile([128, 512], f32, tag="mlp", bufs=2, name="ps_warm")
            nc.tensor.matmul(ps_w[:], ident[:], cbf[:, 0:512])

        def featurize(r):
            C = Cs[r]
            Tp = C * CH
            pk = sp.tile([128, 3, Cmax], f32, tag="pack", bufs=3, name="pk")
            nc.gpsimd.dma_start(out=pk[:, :, 0:C],
                                in_=d_pack[r, :, 0:Tp].rearrange("f (p c) -> p f c", c=C))
            times_sb = pk[:, 0, 0:C]
            vals_sb = pk[:, 1, 0:C]
            measf = pk[:, 2, 0:C]
            iot = sp.tile([128, Cmax], f32, tag="iot", bufs=3, name="iot")
            nc.vector.scalar_tensor_tensor(out=iot[:, 0:C], in0=pbc[:, 0:C],
                                           scalar=float(C), in1=cbc[:, 0:C],
                                           op0=Alu.mult, op1=Alu.add)
            # featurize token-major: xtok [128, C, 33] bf16 (ch 32 = invalid bit)
            xtok = sp.tile([128, Cmax, 33], bf16, tag="xtok", bufs=4, name="xtok")
            nc.vector.tensor_scalar(xtok[:, 0:C, 32:33], iot[:, 0:C].unsqueeze(2),
                                    lensb[:, r:r + 1], None, Alu.is_ge)
            ang = sp.tile([128, Cmax, 8], f32, tag="ang", bufs=3, name="ang")
            nc.vector.tensor_tensor(
                out=ang[:, 0:C, :],
                in0=times_sb.unsqueeze(2).to_broadcast([128, C, 8]),
                in1=itp[:, 0:8].unsqueeze(1).to_broadcast([128, C, 8]),
                op=Alu.mult)
            # round-to-nearest via f32 magic number; frac in [-0.5, 0.5]
            rnd = sp.tile([128, Cmax, 8], f32, tag="rnd", bufs=3, name="rnd")
            BIG = float(1.5 * 2 ** 23)
            nc.vector.tensor_scalar(rnd[:, 0:C, :], ang[:, 0:C, :], BIG, -BIG,
                                    Alu.add, Alu.add)
            nc.vector.tensor_tensor(out=ang[:, 0:C, :], in0=ang[:, 0:C, :],
                                    in1=rnd[:, 0:C, :], op=Alu.subtract)
            # cos half: frac2 = (frac + 0.25) - round(frac + 0.25), also reduced
            nc.vector.tensor_scalar(rnd[:, 0:C, :], ang[:, 0:C, :],
                                    float(0.25) + BIG, -BIG, Alu.add, Alu.add)
            nc.vector.scalar_tensor_tensor(out=rnd[:, 0:C, :], in0=ang[:, 0:C, :],
                                           scalar=0.25, in1=rnd[:, 0:C, :],
                                           op0=Alu.add, op1=Alu.subtract)
            nc.scalar.activation(xtok[:, 0:C, 0:8], ang[:, 0:C, :], Act.Sin,
                                 scale=TWOPI)
            nc.scalar.activation(xtok[:, 0:C, 8:16], rnd[:, 0:C, :], Act.Sin,
                                 scale=TWOPI)
            nc.vector.tensor_copy(xtok[:, 0:C, 16:17], vals_sb.unsqueeze(2))
            nc.vector.tensor_tensor(
                out=xtok[:, 0:C, 17:32],
                in0=measf.unsqueeze(2).to_broadcast([128, C, NMOD]),
                in1=iota15[:].unsqueeze(1).to_broadcast([128, C, NMOD]),
                op=Alu.is_equal)
            return xtok

        def process_mlp(r, xtok):
            C = Cs[r]
            CE = C + 1                      # + demo chunk
            Tp = C * CH
            Text = CE * CH
            NG2 = (CE + 3) // 4             # groups incl demo chunk

            xT = sp.tile([33, (Cmax + 1) * CH], bf16, tag="xT", bufs=2, name="xT")
            nc.gpsimd.memset(xT[0:32, Tp:Text], 0.0)
            nc.gpsimd.memset(xT[32:33, Tp:Text], 1.0)
            nc.vector.tensor_copy(xT[:, Tp:Tp + 1], demoT[0:33, r:r + 1])
            enc_tok = sp.tile([128, Cmax + 1, 132], bf16, tag="enc_tok", bufs=2,
                              name="enc_tok")
            ps_pre = pp.tile([128, Cmax + 1, 4], f32, tag="pre", bufs=2, name="ps_pre")
            for g in range(NG2):
                c0 = g * 4
                nch = min(4, CE - c0)       # chunks incl demo
                ndata = max(0, min(4, C - c0))
                if ndata > 0:
                    pxp = pp.tile([128, 512], bf16, tag="xpose", bufs=2, name="pxp")
                    for j in range(ndata):
                        nc.tensor.transpose(pxp[0:33, j * CH:(j + 1) * CH],
                                            xtok[:, c0 + j, :], ident[:])
                    if g % 2 == 0:
                        nc.vector.tensor_copy(xT[:, c0 * CH:(c0 + ndata) * CH],
                                              pxp[0:33, 0:ndata * CH])
                    else:
                        nc.scalar.copy(xT[:, c0 * CH:(c0 + ndata) * CH],
                                       pxp[0:33, 0:ndata * CH])
                N = nch * CH
                ps1 = pp.tile([128, 512], f32, tag="mlp", bufs=2, name="ps1")
                nc.tensor.matmul(ps1[:, 0:N], w1stk[0:32, :],
                                 xT[0:32, c0 * CH:c0 * CH + N])
                for j in range(nch):
                    nc.tensor.matmul(ps_pre[:, c0 + j, :],
                                     xT[:, (c0 + j) * CH:(c0 + j + 1) * CH],
                                     vxstk[0:33, :])
                h1 = sp.tile([128, 512], bf16, tag="h1", bufs=3, name="h1")
                nc.scalar.activation(h1[:, 0:N], ps1[:, 0:N], Act.Relu, bias=b1)
                ps2 = pp.tile([128, 512], f32, tag="mlp", bufs=2, name="ps2")
                nc.tensor.matmul(ps2[:, 0:N], w2[:], h1[:, 0:N])
                h2 = sp.tile([128, 512], bf16, tag="h2", bufs=3, name="h2")
                nc.vector.tensor_scalar(h2[:, 0:N], ps2[:, 0:N], b2, 0.0,
                                        Alu.add, Alu.max)
                ps3 = pp.tile([128, 4, 128], f32, tag="enc", bufs=2, name="ps3")
                if has_b3:
                    nc.tensor.matmul(ps3[:, 0:nch, :], ones1[:], b3row[:, 0:N],
                                     start=True, stop=False)
                for j in range(nch):
                    nc.tensor.matmul(ps3[:, j, :], h2[:, j * CH:(j + 1) * CH], w3[:],
                                     start=not has_b3, stop=True)
                if g % 2 == 0:
                    nc.scalar.activation(enc_tok[:, c0:c0 + nch, 0:128],
                                         ps3[:, 0:nch, :], Act.Relu)
                else:
                    nc.vector.tensor_scalar(enc_tok[:, c0:c0 + nch, 0:128],
                                            ps3[:, 0:nch, :], 0.0, None, Alu.max)

            return enc_tok, ps_pre

        def process_attn(r, state):
            C = Cs[r]
            CE = C + 1
            enc_tok, ps_pre = state
            # softmax-weighted segment sum + Z via ones channel
            nc.gpsimd.memset(enc_tok[:, 0:CE, 128:129], 1.0)
            e_tok = sp.tile([128, Cmax + 1, H], bf16, tag="e_tok", bufs=2, name="e_tok")
            nc.scalar.activation(e_tok[:, 0:CE, :], ps_pre[:, 0:CE, :], Act.Exp)
            ps_hh = pp.tile([4, 132], f32, tag="pre", bufs=2, name="ps_hh")
            for c in range(CE):
                nc.tensor.matmul(ps_hh[:, 0:129], e_tok[:, c, :], enc_tok[:, c, 0:129],
                                 start=(c == 0), stop=(c == CE - 1))
            rzn = sp.tile([H, 1], f32, tag="rz", bufs=2, name="rzn")
            nc.vector.reciprocal(rzn[:], ps_hh[:, 128:129])
            hh_sb = sp.tile([H, 128], bf16, tag="hh_sb", bufs=2, name="hh_sb")
            nc.vector.tensor_scalar(hh_sb[:], ps_hh[:, 0:128], rzn[:], None, Alu.mult)
            ps_tr = pp.tile([128, 4], bf16, tag="xpose", bufs=2, name="ps_tr")
            nc.tensor.transpose(ps_tr[:], hh_sb[:], ident[0:H, 0:H])
            nc.vector.tensor_copy(feat_all[:, r, :], ps_tr[:])

        # big rows first; featurize one pair ahead; pair exp/head_agg so the
        # scalar engine switches act tables (sin<->exp) once per pair
        feats = {0: featurize(0), 1: featurize(1)}
        for p in range(0, NSLOTS, 2):
            for q in (p + 2, p + 3):
                if q < NSLOTS:
                    feats[q] = featurize(q)
            st0 = process_mlp(p, feats.pop(p))
            st1 = process_mlp(p + 1, feats.pop(p + 1))
            process_attn(p, st0)
            process_attn(p + 1, st1)

        # ---- rho MLP over all 8 rows (f32) ----
        rw1 = bfc("rw1")
        ps_r1 = pp.tile([128, 512], f32, tag="mlp", bufs=2, name="ps_r1")
        for h in range(H):
            nc.tensor.matmul(ps_r1[:, 0:NSLOTS], rw1[:, 128 * h:128 * (h + 1)],
                             feat_all[:, :, h], start=(h == 0), stop=(h == H - 1))
        r1 = sp.tile([128, NSLOTS], f32, tag="r1", name="r1")
        nc.scalar.activation(r1[:], ps_r1[:, 0:NSLOTS], Act.Relu, bias=f32c("rb1"))
        ps_r2 = pp.tile([128, 512], f32, tag="mlp", bufs=2, name="ps_r2")
        nc.tensor.matmul(ps_r2[:, 0:NSLOTS], f32c("rw2"), r1[:])
        r2 = sp.tile([128, NSLOTS], f32, tag="r2", name="r2")
        nc.scalar.activation(r2[:], ps_r2[:, 0:NSLOTS], Act.Relu, bias=f32c("rb2"))
        ps_r3 = pp.tile([4, 512], f32, tag="pre", bufs=2, name="ps_r3")
        nc.tensor.matmul(ps_r3[0:1, 0:NSLOTS], f32c("rw3"), r2[:])
        res = sp.tile([1, NSLOTS], f32, tag="res", name="res")
        nc.scalar.activation(res[:], ps_r3[0:1, 0:NSLOTS], Act.Sigmoid,
                             bias=f32c("rb3")[0:1, :])
        nc.sync.dma_start(out=d_out[:].rearrange("r one -> one r"), in_=res[:])
    return nc


def _patch_tile_drain(tile_mod, mybir):
    """Walrus in this env rejects >1 sync wait per instruction. Two fixes:
    1) split the Tile tail drain's waits across sequential drains;
    2) a post-pass over the final BIR that moves extra waits of ANY
       instruction onto standalone NoOps inserted just before it."""
    from concourse.vector_clock import ScopedClock
    if getattr(tile_mod.TileContext, "_drain_patched", False):
        return

    def _drain_and_barrier(self, tick_clock, wait_clock):
        nc = self.nc
        drain_inst = nc.sync.drain()
        wait_clock.add_sem_waits(drain_inst.ins, ScopedClock({None: tick_clock.global_clock}))
        si = drain_inst.ins.sync_info
        waits = list(si.on_wait or [])
        if len(waits) > 1:
            si.on_wait = waits[:1]
            for i in range(1, len(waits)):
                extra = nc.sync.drain()
                esi = extra.ins.sync_info
                if esi is None:
                    extra.ins.sync_info = mybir.SyncInfo(on_wait=waits[i:i + 1], on_update=[])
                else:
                    esi.on_wait = waits[i:i + 1]
        nc.all_engine_barrier()
        popped = nc._tile_sem_poison_stack.pop()
        assert popped is self._sem_poison
        nc.clear_and_free_semaphores(list(self.sems.allocated().values()))
        nc.all_engine_barrier()

    tile_mod.TileContext._drain_and_barrier = _drain_and_barrier

    _orig_exit = tile_mod.TileContext.__exit__

    def _exit(self, exc_type, exc_val, exc_tb):
        r = _orig_exit(self, exc_type, exc_val, exc_tb)
        if exc_type is None and getattr(tile_mod.TileContext, "_split_waits", True):
            _split_multi_waits(self.nc, mybir)
        return r

    def _split_multi_waits(nc, mybir):
        n = [0]
        for f in nc.m.functions:
            for bb in f.blocks:
                insts = bb.instructions
                out = []
                for inst in insts:
                    si = inst.sync_info
                    waits = list(si.on_wait) if (si and si.on_wait) else []
                    if len(waits) > 1:
                        for w in waits[:-1]:
                            n[0] += 1
                            nop = mybir.InstNoOp(name=f"I-ws-{n[0]}", ins=[], outs=[])
                            nop.engine = inst.engine
                            nop.sync_info = mybir.SyncInfo(on_wait=[w], on_update=[])
                            out.append(nop)
                        si.on_wait = waits[-1:]
                    out.append(inst)
                if len(out) != len(insts):
                    bb.instructions = out

    tile_mod.TileContext.__exit__ = _exit
    tile_mod.TileContext._drain_patched = True


_CACHE = {}
last_results = None


def _maybe_install_ntff_shim():
    """The image's antenv lacks axon_hooks; register the ctypes NTFF hook so
    run_bass_kernel_spmd(trace=True) can profile."""
    import sys, types
    if "antenv.axon_hooks" in sys.modules:
        return
    try:
        from trn_agent_boot.trn_boot import _ntff_profile_via_ctypes
        hook = _ntff_profile_via_ctypes("/opt/axon/libaxon_pjrt.so")
    except Exception:
        hook = None
    mod = types.ModuleType("antenv.axon_hooks")
    mod.get_axon_ntff_profile_hook = lambda: hook
    sys.modules["antenv.axon_hooks"] = mod


def _to_bf16(a):
    import ml_dtypes
    return np.asarray(a, np.float32).astype(ml_dtypes.bfloat16)


def kernel(**inputs):
    import os
    import concourse.bass as bass
    import concourse.mybir as mybir
    import concourse.tile as tile_mod
    from concourse import bass_utils

    _patch_tile_drain(tile_mod, mybir)

    inp = {k: np.asarray(v) for k, v in inputs.items()}
    pack = np.stack([inp["times"].astype(np.float32)[..., 0],
                     inp["values"].astype(np.float32)[..., 0],
                     inp["measurements"].astype(np.float32)], axis=1)         # [B,3,T]
    pack = np.ascontiguousarray(pack)
    lengths = inp["lengths"].astype(np.int64)                                # [B]
    demo = inp["demo"].astype(np.float32)

    order = np.argsort(-lengths, kind="stable")
    Cs = []
    for s in range(NSLOTS):
        ranks = order[s * NCORES:(s + 1) * NCORES]
        Cs.append(int(np.ceil(lengths[ranks].max() / CH)))

    b3 = inp["phi_b3"].astype(np.float32)
    has_b3 = bool(np.abs(b3).max() > 0)

    key = (tuple(Cs), has_b3)
    if key not in _CACHE:
        _CACHE[key] = _build_nc(Cs, has_b3, tile_mod, bass, mybir)
    nc = _CACHE[key]

    itp, iota15, pbc, cbc, p0, ident = _host_constants()

    # host: Vx and demo-token phi-input features
    Wk32 = inp["W_k"].astype(np.float32)[0:32].reshape(32, H, DP)
    Vx = np.einsum('fhd,hd->fh', Wk32, inp["W_q"].astype(np.float32)) / np.sqrt(DP)
    vxstk = np.concatenate(
        [Vx, np.full((1, H), -1e9, np.float32),
         np.zeros((95, H), np.float32)], 0)       # [128,4]; row 32 = mask wt
    w1stk = np.tile(inp["phi_w1"].astype(np.float32), (4, 1))                 # [128,128]
    demo_enc = (np.maximum(demo @ inp["demo_w1"].astype(np.float32)
                           + inp["demo_b1"].astype(np.float32), 0.0)
                @ inp["demo_w2"].astype(np.float32)
                + inp["demo_b2"].astype(np.float32))                          # [B,32]

    def blob(cols, parts):
        n = sum(w for _, w in cols)
        a = np.zeros((128, n), np.float32)
        o = 0
        for name, w in cols:
            v = parts[name]
            a[:v.shape[0], o:o + w] = v
            o += w
        return a

    f32_parts = {
        "itp": itp, "pbc": pbc, "cbc": cbc, "iota15": iota15, "p0": p0,
        "b1": inp["phi_b1"].astype(np.float32)[:, None],
        "b2": inp["phi_b2"].astype(np.float32)[:, None],
        "rb1": inp["rho_b1"].astype(np.float32)[:, None],
        "rb2": inp["rho_b2"].astype(np.float32)[:, None],
        "rb3": np.broadcast_to(inp["rho_b3"].astype(np.float32)[:, None], (128, 1)),
        "rw2": inp["rho_w2"].astype(np.float32),
        "rw3": inp["rho_w3"].astype(np.float32),
    }
    bf_parts_common = {
        "ident": ident, "w1stk": w1stk, "w2": inp["phi_w2"].astype(np.float32),
        "w3": inp["phi_w3"].astype(np.float32),
        "rw1": inp["rho_w1"].astype(np.float32).reshape(4, 128, 128)
              .transpose(1, 0, 2).reshape(128, 512),
        "vxstk": vxstk,
    }

    in_maps = []
    for core in range(NCORES):
        rows = [order[s * NCORES + core] for s in range(NSLOTS)]
        f32_parts["lens"] = np.broadcast_to(
            lengths[rows].astype(np.float32)[None, :], (128, NSLOTS))
        bf_parts = dict(bf_parts_common)
        bf_parts["demoT"] = np.concatenate(
            [demo_enc[rows].T, np.zeros((96, NSLOTS), np.float32)], 0)        # [128,8]
        m = {
            "pack_r": pack[rows],
            "cf32": blob(F32_COLS, f32_parts),
            "cbf": _to_bf16(blob(BF_COLS, bf_parts)),
        }
        if has_b3:
            m["b3row"] = _to_bf16(np.tile(b3[None, :], (1, 4)))
            m["ones1"] = _to_bf16(np.ones((1, 128), np.float32))
        in_maps.append(m)

    trace = os.environ.get("KERNEL_TRACE", "0") == "1"
    kw = {}
    if trace:
        _maybe_install_ntff_shim()
        kw = dict(trace=True, tmpdir=os.environ.get("KERNEL_TRACE_DIR") or None)
    res = bass_utils.run_bass_kernel_spmd(nc, in_maps, core_ids=list(range(NCORES)), **kw)
    global last_results
    last_results = res
    out = np.zeros((B, 1), np.float32)
    for core in range(NCORES):
        for s in range(NSLOTS):
            out[order[s * NCORES + core], 0] = res.results[core]["out"][s, 0]
    return out
